# revision 3
# baseline (speedup 1.0000x reference)
"""DSQG block (sparse attention + gated out-proj + SwiGLU FFN) on 8 TRN2 cores.

Sharding: attention is head-parallel (2 heads/core, all 2048 rows); the
out-proj + FFN are row-parallel (256 rows/core).  The two halves are bridged
by one AllToAll of the gated attention output (0.5 MB/rank in bf16).

v2 layout (bf16 everywhere on the matmul paths, fp32 accumulation):
  - all heavy matmuls take bf16 inputs -> 4x PE throughput vs fp32, and all
    weights stream from DRAM in bf16 -> half the HBM traffic.
  - rmsnorm1 folded: norm1_scale into weights (host), 1/rms applied per
    partition on the qkv/gate matmul output rows.
  - near scores for offsets {0..32,48,64,96,128,192} via ONE (128,384) PE band
    matmul per (tile, head) -> DRAM bounce -> 3 strided diagonal-gather DMAs.
  - far offsets {256,...,1536} are all multiples of 128: pure SBUF tile
    reindex (no shifted reload), scores via DVE mul+reduce.
  - near AV: alphas scattered (transposed diagonal AP DMA) into a (384,128)
    DRAM W matrix -> 3 accumulating PE matmuls against v tiles t-2,t-1,t.
    No PE transposes of W needed.
  - far AV via chained scalar_tensor_tensor on DVE with fp32 accumulator.
"""

import sys

for _p in ("/opt/trn_rl_repo",):
    if _p not in sys.path:
        sys.path.insert(0, _p)

import numpy as np
import ml_dtypes

BF16NP = np.dtype(ml_dtypes.bfloat16)

B, N, D, H, FFN = 1, 2048, 1024, 16, 2816
HD = D // H          # 64
NCORES = 8
NT = N // 128        # 16 row tiles
KD = D // 128        # 8 contraction tiles
ROWS = N // NCORES   # 256 rows per core for the FFN half
OFFS = sorted(set(range(0, 33)) | {48, 64, 96, 128, 192, 256, 384, 512, 768, 1024, 1536})
# score-column layout: band-resolved offsets first (row position in the W
# scatter ascends with column), then far offsets.
BAND_COLS = [192, 128, 96, 80, 64, 48] + list(range(32, -1, -1))  # 39 cols; 80 is a dummy
FAR = [256, 384, 512, 768, 1024, 1536]
COLS = BAND_COLS + FAR
NO = len(COLS)       # 45
NEG = -30000.0


_CACHE = {}


def _build():
    import concourse.bass as bass
    import concourse.mybir as mybir
    from concourse import bacc
    from concourse.tile import TileContext

    F32 = mybir.dt.float32
    BF = mybir.dt.bfloat16
    AF = mybir.ActivationFunctionType
    OP = mybir.AluOpType
    AX = mybir.AxisListType

    nc = bacc.Bacc("TRN2", target_bir_lowering=False, debug=False, num_devices=NCORES)

    P = {}
    def par(name, shape, dt):
        P[name] = nc.declare_dram_parameter(name, list(shape), dt, isOutput=False)
        return P[name]

    xT = par("xT", (D, N), BF)
    xres = par("xres", (ROWS, D), F32)
    wqkvg = par("wqkvg", (D, 512), BF)
    w_out = par("w_out", (D, D), BF)
    wgu = par("wgu", (D, 2 * FFN), BF)
    wdn = par("wdn", (FFN, D), BF)
    bgate = par("bgate", (128, 128), BF)
    pm = par("pm", (128, NT, 2, NO), BF)
    ident_in = par("ident", (128, 128), BF)
    y = nc.declare_dram_parameter("y", [ROWS, D], F32, isOutput=True)

    QC, KC, VC, GC = slice(0, 128), slice(128, 256), slice(256, 384), slice(384, 512)

    with TileContext(nc) as tc:
      with (
        tc.tile_pool(name="const", bufs=1) as cp,
        tc.tile_pool(name="dramp", bufs=1, space="DRAM") as dp,
      ):
        pp = tc.alloc_tile_pool(name="persist", bufs=1)
        ident = cp.tile([128, 128], BF)
        nc.sync.dma_start(ident[:], ident_in.ap())
        bg = cp.tile([128, 128], BF)
        nc.sync.dma_start(bg[:], bgate.ap())
        pmt = cp.tile([128, NT, 2, NO], BF)
        nc.sync.dma_start(pmt[:], pm.ap())

        # persistent activation buffers.  qkv tile index t+2 <-> row tile t;
        # indices 0,1 are zero tiles (band/AV windows read tiles t-2,t-1).
        qkv = pp.tile([128, NT + 2, 384], BF)
        gateb = pp.tile([128, NT, 128], BF)
        qT2 = pp.tile([128, N], BF)                 # (d2, n) transposed q (pre-scaled 1/8)
        kT2 = pp.tile([128, 256 + N], BF)           # zero prefix of 256 cols
        S_all = pp.tile([128, NT, 2, NO], BF)
        A_all = pp.tile([128, NT, 2, NO], BF)
        A_far = pp.tile([128, NT, 2, len(FAR)], F32)
        ssum = pp.tile([128, NT, 2], F32)
        rec = pp.tile([128, NT, 2], F32)
        ss_all = pp.tile([128, NT], F32)
        rrms = pp.tile([128, NT], F32)
        navs = pp.tile([128, NT, 2, 64], F32)
        acc_all = pp.tile([128, NT, 2, 64], F32)

        nc.gpsimd.memset(qkv[:, 0:2, :], 0.0)
        nc.gpsimd.memset(kT2[:, 0:256], 0.0)
        nc.gpsimd.memset(S_all[:], 0.0)

        cc_in = dp.tile([N, 128], BF, tag="cc_in")
        cc_out = dp.tile([N, 128], BF, tag="cc_out")

        # near-AV scatter targets in DRAM, zeroed once, re-scattered in place
        NWBUF = 8
        wnear = [dp.tile([384, 128], BF, tag=f"wnear{j}", name=f"wnear{j}") for j in range(NWBUF)]
        epsb = cp.tile([128, 1], F32)
        nc.gpsimd.memset(epsb[:], 1e-6)
        ztw = cp.tile([128, 384], BF)
        nc.gpsimd.memset(ztw[:], 0.0)
        for j in range(NWBUF):
            dst = bass.AP(tensor=wnear[j].tensor, offset=wnear[j].offset,
                          ap=[[384, 128], [1, 384]])
            nc.sync.dma_start(dst, ztw[:])

        # ---------- phase B: fused qkv+gate matmul, rrms on eviction ----------
        with (
            tc.tile_pool(name="qph", bufs=1) as qp,
        ):
            psR = tc.alloc_tile_pool(name="psR", bufs=1, space="PSUM")
            wq = qp.tile([128, KD, 512], BF)
            nc.sync.dma_start(wq[:], wqkvg.ap().rearrange("(k p) c -> p k c", p=128))
            xts = qp.tile([128, KD, N], BF)
            for k in range(KD):
                nc.sync.dma_start(xts[:, k, :], xT.ap()[k * 128:(k + 1) * 128, :])
            # rmsnorm1 stats from xT: sumsq over d via ones-matmul on squared tiles
            ones = cp.tile([128, 1], BF)
            nc.gpsimd.memset(ones[:], 1.0)
            pss = [psR.tile([1, 512], F32, tag=f"pss{j}", bufs=1, name=f"pss{j}")
                   for j in range(4)]
            for k in range(KD):
                xsq = qp.tile([128, N], BF, tag="xsq", bufs=2)
                nc.scalar.activation(xsq[:], xts[:, k, :], AF.Square)
                for j in range(4):
                    nc.tensor.matmul(pss[j][:], ones[:], xsq[:, j * 512:(j + 1) * 512],
                                     start=(k == 0), stop=(k == KD - 1))
            ssrow = qp.tile([1, N], F32)
            for j in range(4):
                nc.vector.tensor_copy(ssrow[:, j * 512:(j + 1) * 512], pss[j][:])
            ss_dram = dp.tile([1, N], F32, tag="ss_dram")
            nc.sync.dma_start(ss_dram[:], ssrow[:])
            nc.sync.dma_start(
                ss_all[:],
                bass.AP(tensor=ss_dram.tensor, offset=ss_dram.offset,
                        ap=[[1, 128], [128, NT]]))
            srt = cp.tile([128, NT], F32)
            nc.scalar.activation(srt[:], ss_all[:], AF.Sqrt, scale=1.0 / D, bias=epsb[:])
            nc.vector.reciprocal(rrms[:], srt[:])
            psR.release()
            psA = tc.alloc_tile_pool(name="psA", bufs=4, space="PSUM")
            psT = tc.alloc_tile_pool(name="psT", bufs=2, space="PSUM")
            for t in range(NT):
                ps = psA.tile([128, 512], F32, tag="qkvg_ps")
                for k in range(KD):
                    nc.tensor.matmul(ps[:], xts[:, k, t * 128:(t + 1) * 128],
                                     wq[:, k, :], start=(k == 0), stop=(k == KD - 1))
                rr = rrms[:, t:t + 1]
                # q gets the extra 1/sqrt(HD) score scale
                nc.vector.tensor_scalar(qkv[:, t + 2, QC], ps[:, QC], rr,
                                        float(HD) ** -0.5, OP.mult, OP.mult)
                nc.vector.tensor_scalar(qkv[:, t + 2, 128:384], ps[:, 128:384],
                                        rr, None, OP.mult)
                nc.vector.tensor_scalar(gateb[:, t, :], ps[:, GC], rr, None, OP.mult)
                # transposes of q and k for the near-band matmuls
                pq = psT.tile([128, 128], BF, tag="tq")
                nc.tensor.transpose(pq[:], qkv[:, t + 2, QC], ident[:])
                nc.scalar.activation(qT2[:, t * 128:(t + 1) * 128], pq[:], AF.Copy)
                pk = psT.tile([128, 128], BF, tag="tk")
                nc.tensor.transpose(pk[:], qkv[:, t + 2, KC], ident[:])
                nc.scalar.activation(kT2[:, 256 + t * 128:256 + (t + 1) * 128], pk[:], AF.Copy)
            psT.release()
            psA.release()

        # ---------- phase E: band scores (PE band mm -> DRAM diag gather) ----------
        # band covers key cols [t*128-256, t*128+128): S[i,o] at sd[i, 256+i-o]
        with (
            tc.tile_pool(name="nearp", bufs=4, space="PSUM") as psS,
            tc.tile_pool(name="neard", bufs=4, space="DRAM") as ndp,
            tc.tile_pool(name="neard_sb", bufs=4) as ndp_sb,
        ):
            for t in range(NT):
                for h in range(2):
                    ps = psS.tile([128, 384], F32, tag="sd")
                    nc.tensor.matmul(ps[:], qT2[64 * h:64 * h + 64, t * 128:(t + 1) * 128],
                                     kT2[64 * h:64 * h + 64, t * 128:t * 128 + 384],
                                     start=True, stop=True)
                    sd_sb = ndp_sb.tile([128, 384], BF, tag="sd_sb", bufs=4)
                    nc.scalar.activation(sd_sb[:], ps[:], AF.Copy)
                    sd = ndp.tile([128, 384], BF, tag="sdd")
                    nc.sync.dma_start(sd[:], sd_sb[:])
                    # 3 diagonal gathers: cols {192,128}, {96,80,64,48}, {32..0}
                    nc.sync.dma_start(
                        S_all[:, t, h, 0:2],
                        bass.AP(tensor=sd.tensor, offset=sd.offset + 64,
                                ap=[[385, 128], [64, 2]]))
                    nc.sync.dma_start(
                        S_all[:, t, h, 2:6],
                        bass.AP(tensor=sd.tensor, offset=sd.offset + 160,
                                ap=[[385, 128], [16, 4]]))
                    nc.sync.dma_start(
                        S_all[:, t, h, 6:39],
                        bass.AP(tensor=sd.tensor, offset=sd.offset + 224,
                                ap=[[385, 128], [1, 33]]))

            # ---------- phase F: far scores (SBUF tile reindex + DVE) ----------
            with tc.tile_pool(name="farp", bufs=2) as fp_:
                for oi, o in enumerate(FAR):
                    col = 39 + oi
                    s = o // 128
                    ntl = NT - s
                    tmp = fp_.tile([128, NT, 128], BF, tag="ftmp")
                    meng = nc.gpsimd if (oi % 2 == 1) else nc.vector
                    meng.tensor_mul(tmp[:, 0:ntl, :],
                                    qkv[:, s + 2:NT + 2, QC],
                                    qkv[:, 2:NT + 2 - s, KC])
                    red_in = tmp[:, 0:ntl, :].rearrange("p t (h d) -> p t h d", h=2)
                    with nc.allow_low_precision(reason="scores tolerate bf16"):
                        nc.vector.tensor_reduce(S_all[:, s:NT, :, col],
                                                red_in, AX.X, OP.add)

            # ---------- phase G: softmax (no max-sub; scores are bounded) ----------
            nc.vector.tensor_add(S_all[:], S_all[:], pmt[:])
            nc.scalar.activation(A_all[:], S_all[:], AF.Exp)
            nc.vector.tensor_reduce(ssum[:], A_all[:], AX.X, OP.add)
            nc.vector.reciprocal(rec[:], ssum[:])
            nc.vector.tensor_copy(A_far[:], A_all[:, :, :, 39:39 + len(FAR)])

            # ---------- phase H: AV + gate + og ----------
            with (
                tc.tile_pool(name="avp", bufs=8) as avp,
                tc.tile_pool(name="avps", bufs=4, space="PSUM") as psAV,
                tc.tile_pool(name="ogp", bufs=4) as ogp,
            ):
                for t in range(NT):
                    for h in range(2):
                        vc = slice(256 + 64 * h, 256 + 64 * h + 64)
                        # W (384,128): row r = key (t-2)*128+r, col i = query i.
                        # scatter A[i, col] to row i + 256 - o, col i.
                        wd = wnear[(t * 2 + h) % NWBUF]
                        nc.sync.dma_start(
                            bass.AP(tensor=wd.tensor, offset=wd.offset + 64 * 128,
                                    ap=[[129, 128], [64 * 128, 2]]),
                            A_all[:, t, h, 0:2])
                        nc.sync.dma_start(
                            bass.AP(tensor=wd.tensor, offset=wd.offset + 160 * 128,
                                    ap=[[129, 128], [16 * 128, 4]]),
                            A_all[:, t, h, 2:6])
                        nc.sync.dma_start(
                            bass.AP(tensor=wd.tensor, offset=wd.offset + 224 * 128,
                                    ap=[[129, 128], [128, 33]]),
                            A_all[:, t, h, 6:39])
                        wA = avp.tile([128, 128], BF, tag="wA")
                        wB = avp.tile([128, 128], BF, tag="wB")
                        wC = avp.tile([128, 128], BF, tag="wC")
                        nc.sync.dma_start(wA[:], wd[0:128, :])
                        nc.sync.dma_start(wB[:], wd[128:256, :])
                        nc.sync.dma_start(wC[:], wd[256:384, :])
                        pav = psAV.tile([128, 64], F32, tag="pav", bufs=2)
                        nc.tensor.matmul(pav[:], wA[:], qkv[:, t, vc],
                                         start=True, stop=False)
                        nc.tensor.matmul(pav[:], wB[:], qkv[:, t + 1, vc],
                                         start=False, stop=False)
                        nc.tensor.matmul(pav[:], wC[:], qkv[:, t + 2, vc],
                                         start=False, stop=True)
                        nc.scalar.activation(navs[:, t, h, :], pav[:], AF.Copy)

                # far AV: offset-outer, SBUF tile reindex, chained STT per (t,h)
                nc.vector.memset(acc_all[:], 0.0)
                for oi, o in enumerate(FAR):
                    s = o // 128
                    for t in range(s, NT):
                        for h in range(2):
                            nc.vector.scalar_tensor_tensor(
                                acc_all[:, t, h, :],
                                qkv[:, t + 2 - s, 256 + 64 * h:256 + 64 * h + 64],
                                A_far[:, t, h, oi:oi + 1],
                                acc_all[:, t, h, :], OP.mult, OP.add)

                for t in range(NT):
                    gt = ogp.tile([128, 128], F32, tag="gate")
                    gtr = ogp.tile([128, 128], BF, tag="gtr")
                    nc.vector.tensor_add(gtr[:], gateb[:, t, :], bg[:])
                    nc.scalar.activation(gt[:], gtr[:], AF.Sigmoid)
                    og = ogp.tile([128, 128], BF, tag="og")
                    for h in range(2):
                        comb = avp.tile([128, 64], F32, tag="comb")
                        nc.gpsimd.tensor_add(comb[:], navs[:, t, h, :], acc_all[:, t, h, :])
                        nc.vector.scalar_tensor_tensor(
                            og[:, 64 * h:64 * h + 64], comb[:],
                            rec[:, t, h:h + 1], gt[:, 64 * h:64 * h + 64],
                            OP.mult, OP.mult)
                    nc.sync.dma_start(cc_in[t * 128:(t + 1) * 128, :], og[:])

        # ---------- phase I: AllToAll + assemble own 256 rows ----------
        pp.release()
        nc.gpsimd.collective_compute(
            "AllToAll", mybir.AluOpType.bypass,
            replica_groups=[list(range(NCORES))],
            ins=[cc_in.opt()], outs=[cc_out.opt()],
        )

        with (
            tc.tile_pool(name="oproj", bufs=1) as op_,
        ):
            psO = tc.alloc_tile_pool(name="psO", bufs=2, space="PSUM")
            psT2 = tc.alloc_tile_pool(name="psT2", bufs=1, space="PSUM")
            ogf = op_.tile([128, 2, D], BF)      # (n-part, nb, d2)
            for r in range(NCORES):
                for b in range(2):
                    nc.sync.dma_start(ogf[:, b, r * 128:(r + 1) * 128],
                                      cc_out[r * ROWS + b * 128:r * ROWS + (b + 1) * 128, :])
            ogfT = op_.tile([128, KD, ROWS], BF)  # (d2-part, k, n)
            for b in range(2):
                for k in range(KD):
                    pt = psT2.tile([128, 128], BF, tag="ot")
                    nc.tensor.transpose(pt[:], ogf[:, b, k * 128:(k + 1) * 128], ident[:])
                    nc.scalar.activation(ogfT[:, k, b * 128:(b + 1) * 128], pt[:], AF.Copy)

            wo = op_.tile([128, KD, D], BF)
            nc.sync.dma_start(wo[:], w_out.ap().rearrange("(k p) c -> p k c", p=128))
            x2 = op_.tile([128, 2, D], F32)
            xr = op_.tile([128, 2, D], F32)
            nc.sync.dma_start(xr[:], xres.ap().rearrange("(b p) c -> p b c", p=128))
            for b in range(2):
                for half in range(2):
                    ps = psO.tile([128, 512], F32, tag="ops")
                    cs = slice(half * 512, (half + 1) * 512)
                    for k in range(KD):
                        nc.tensor.matmul(ps[:], ogfT[:, k, b * 128:(b + 1) * 128],
                                         wo[:, k, cs], start=(k == 0), stop=(k == KD - 1))
                    nc.vector.tensor_add(x2[:, b, cs], ps[:], xr[:, b, cs])

            # ---------- norm2 + transpose ----------
            ss2 = op_.tile([128, 2], F32)
            for b in range(2):
                sq2 = op_.tile([128, D], F32, tag="sq2", bufs=2)
                nc.scalar.activation(sq2[:], x2[:, b, :], AF.Square,
                                     accum_out=ss2[:, b:b + 1])
            srt2 = op_.tile([128, 2], F32)
            nc.scalar.activation(srt2[:], ss2[:], AF.Sqrt, scale=1.0 / D, bias=epsb[:])
            rr2 = op_.tile([128, 2], F32)
            nc.vector.reciprocal(rr2[:], srt2[:])
            xn2 = op_.tile([128, 2, D], BF)
            for b in range(2):
                nc.vector.tensor_scalar(xn2[:, b, :], x2[:, b, :], rr2[:, b:b + 1],
                                        None, OP.mult)
            xn2T = op_.tile([128, KD, ROWS], BF)
            for b in range(2):
                for k in range(KD):
                    pt = psT2.tile([128, 128], BF, tag="xt2")
                    nc.tensor.transpose(pt[:], xn2[:, b, k * 128:(k + 1) * 128], ident[:])
                    nc.scalar.activation(xn2T[:, k, b * 128:(b + 1) * 128], pt[:], AF.Copy)

            # ---------- FFN ----------
            psT2.release()
            psO.release()
            FT = FFN // 128  # 22
            with (
                tc.tile_pool(name="ffnw", bufs=3) as fw,
                tc.tile_pool(name="ffnh", bufs=1) as fh,
                tc.tile_pool(name="psF", bufs=1, space="PSUM") as psF,
            ):
                hT = fh.tile([128, FT, ROWS], BF)
                for m in range(FT):
                    wg_m = fw.tile([128, KD, 128], BF, tag="wg")
                    nc.sync.dma_start(
                        wg_m[:], wgu.ap()[:, m * 128:(m + 1) * 128]
                        .rearrange("(k p) c -> p k c", p=128))
                    wu_m = fw.tile([128, KD, 128], BF, tag="wu")
                    nc.sync.dma_start(
                        wu_m[:], wgu.ap()[:, FFN + m * 128:FFN + (m + 1) * 128]
                        .rearrange("(k p) c -> p k c", p=128))
                    pg = psF.tile([128, ROWS], F32, tag="pg")
                    pu = psF.tile([128, ROWS], F32, tag="pu")
                    for k in range(KD):
                        nc.tensor.matmul(pg[:], wg_m[:, k, :], xn2T[:, k, :],
                                         start=(k == 0), stop=(k == KD - 1))
                    for k in range(KD):
                        nc.tensor.matmul(pu[:], wu_m[:, k, :], xn2T[:, k, :],
                                         start=(k == 0), stop=(k == KD - 1))
                    sg = fw.tile([128, ROWS], F32, tag="sg", bufs=2)
                    nc.scalar.activation(sg[:], pg[:], AF.Silu)
                    nc.vector.tensor_mul(hT[:, m, :], sg[:], pu[:])

                out_sb = fh.tile([128, 2, D], F32)
                pds = [psF.tile([128, 512], F32, tag=f"pd{j}", bufs=1, name=f"pd{j}")
                       for j in range(4)]
                for k2 in range(FT):
                    wd_k = fw.tile([128, D], BF, tag="wdk")
                    nc.sync.dma_start(wd_k[:], wdn.ap()[k2 * 128:(k2 + 1) * 128, :])
                    for b in range(2):
                        for half in range(2):
                            nc.tensor.matmul(
                                pds[b * 2 + half][:],
                                hT[:, k2, b * 128:(b + 1) * 128],
                                wd_k[:, half * 512:(half + 1) * 512],
                                start=(k2 == 0), stop=(k2 == FT - 1))
                for b in range(2):
                    for half in range(2):
                        cs = slice(half * 512, (half + 1) * 512)
                        nc.vector.tensor_add(out_sb[:, b, cs], pds[b * 2 + half][:],
                                             x2[:, b, cs])
                for b in range(2):
                    nc.sync.dma_start(y.ap()[b * 128:(b + 1) * 128, :], out_sb[:, b, :])

    nc.finalize()
    return nc


def _host_prep(inputs):
    x = np.asarray(inputs["x"], np.float32)
    n1 = np.asarray(inputs["norm1_scale"], np.float32)
    n2 = np.asarray(inputs["norm2_scale"], np.float32)
    w_qkv = np.asarray(inputs["w_qkv"], np.float32)
    w_out = np.asarray(inputs["w_out"], np.float32)
    w_gate = np.asarray(inputs["w_gate"], np.float32)
    b_gate = np.asarray(inputs["b_gate"], np.float32)
    pos_bias = np.asarray(inputs["pos_bias"], np.float32)
    w_fg = np.asarray(inputs["w_ffn_gate"], np.float32)
    w_fu = np.asarray(inputs["w_ffn_up"], np.float32)
    w_fd = np.asarray(inputs["w_ffn_down"], np.float32)
    offs = np.asarray(inputs["offsets"], np.int64)
    assert list(offs) == OFFS, "offset set changed; kernel segmentation is stale"

    x2d = np.ascontiguousarray(x.reshape(N, D))
    xT = np.ascontiguousarray(x2d.T.astype(BF16NP))
    wgu = np.ascontiguousarray((np.concatenate([w_fg, w_fu], axis=1)
                                * n2[:, None]).astype(BF16NP))
    wdn_b = np.ascontiguousarray(w_fd.astype(BF16NP))
    w_out_b = np.ascontiguousarray(w_out.astype(BF16NP))
    ident = np.eye(128, dtype=BF16NP)
    wq_s = w_qkv * n1[:, None]
    wg_s = w_gate * n1[:, None]

    tvec = np.arange(N).reshape(NT, 128)

    in_maps = []
    for c in range(NCORES):
        h0, h1 = 2 * c, 2 * c + 1
        cols = []
        for sec in range(3):  # q, k, v
            for h in (h0, h1):
                cols.append(wq_s[:, sec * D + h * HD: sec * D + (h + 1) * HD])
        cols.append(wg_s[:, c * 128:(c + 1) * 128])
        wqkvg = np.ascontiguousarray(np.concatenate(cols, axis=1).astype(BF16NP))

        pmc = np.full((128, NT, 2, NO), NEG, np.float32)
        for hh, h in enumerate((h0, h1)):
            for ci, o in enumerate(COLS):
                if o not in OFFS:
                    continue  # dummy column stays NEG
                o_i = OFFS.index(o)
                valid = (tvec >= o)  # (NT, 128)
                pmc[:, :, hh, ci] = np.where(valid.T, pos_bias[o_i, h], NEG)
        bgate_b = np.broadcast_to(b_gate[c * 128:(c + 1) * 128], (128, 128))

        in_maps.append({
            "xT": xT,
            "xres": np.ascontiguousarray(x2d[c * ROWS:(c + 1) * ROWS]),
            "wqkvg": wqkvg,
            "w_out": w_out_b,
            "wgu": wgu,
            "wdn": wdn_b,
            "bgate": np.ascontiguousarray(bgate_b.astype(BF16NP)),
            "pm": np.ascontiguousarray(pmc.astype(BF16NP)),
            "ident": ident,
        })
    return in_maps


def _get_nc():
    if "nc" not in _CACHE:
        _CACHE["nc"] = _build()
    return _CACHE["nc"]


def kernel(**inputs) -> np.ndarray:
    from concourse import bass_utils
    nc = _get_nc()
    in_maps = _host_prep(inputs)
    res = bass_utils.run_bass_kernel_spmd(
        nc, in_maps, core_ids=list(range(NCORES)), trace=False)
    y = np.concatenate([res.results[c]["y"] for c in range(NCORES)], axis=0)
    return y.reshape(B, N, D).astype(np.float32)


# keep a handle for test.py to run with tracing
def run_traced(inputs, tmpdir=None):
    from concourse import bass_utils
    nc = _get_nc()
    in_maps = _host_prep(inputs)
    res = bass_utils.run_bass_kernel_spmd(
        nc, in_maps, core_ids=list(range(NCORES)), trace=True, tmpdir=tmpdir)
    y = np.concatenate([res.results[c]["y"] for c in range(NCORES)], axis=0)
    return y.reshape(B, N, D).astype(np.float32), res


# revision 18
# speedup vs baseline: 3.3314x; 3.3314x over previous
"""DSQG block (sparse attention + gated out-proj + SwiGLU FFN) on 8 TRN2 cores.

Sharding: attention is head-parallel (2 heads/core, all 2048 rows); the
out-proj + FFN are row-parallel (256 rows/core).  The two halves are bridged
by one AllToAll of the gated attention output (0.5 MB/rank in bf16).

v3 layout (bf16 matmul paths, fp32 accumulation, no DRAM bounces):
  - all heavy matmuls take bf16 inputs -> 4x PE throughput vs fp32, and all
    weights stream from DRAM in bf16 -> half the HBM traffic.
  - rmsnorm1 folded: norm1_scale into weights (host), 1/rms applied per
    partition on the qkv/gate matmul output rows.
  - near offsets {0..32,48,64,96,128,192}: band scores computed TRANSPOSED
    (key-row major) via 3 chunked PE matmuls per (tile, head); the bias+mask
    pmT is host-built in the same orientation; alphas stay in SBUF and feed
    the AV matmuls directly (v tiles t-2,t-1,t as stationary).  The softmax
    denominator comes from a ones-matmul partition reduce.
  - far offsets {256..1536} are all multiples of 128: pure SBUF tile
    reindex, scores via gpsimd mul+reduce, AV via chained STT on gpsimd.
  - the only DRAM round trips left: rms-stat broadcast and the collective.
"""

import sys

for _p in ("/opt/trn_rl_repo",):
    if _p not in sys.path:
        sys.path.insert(0, _p)

import numpy as np
import ml_dtypes

BF16NP = np.dtype(ml_dtypes.bfloat16)

B, N, D, H, FFN = 1, 2048, 1024, 16, 2816
HD = D // H          # 64
NCORES = 8
NT = N // 128        # 16 row tiles
KD = D // 128        # 8 contraction tiles
ROWS = N // NCORES   # 256 rows per core for the FFN half
OFFS = sorted(set(range(0, 33)) | {48, 64, 96, 128, 192, 256, 384, 512, 768, 1024, 1536})
BANDSET = set(range(0, 33)) | {48, 64, 96, 128, 192}
FAR = [256, 384, 512, 768, 1024, 1536]
NFAR = len(FAR)
NEG = -30000.0


_CACHE = {}


def _build():
    import concourse.bass as bass
    import concourse.mybir as mybir
    from concourse import bacc
    from concourse.tile import TileContext

    F32 = mybir.dt.float32
    BF = mybir.dt.bfloat16
    AF = mybir.ActivationFunctionType
    OP = mybir.AluOpType
    AX = mybir.AxisListType

    nc = bacc.Bacc("TRN2", target_bir_lowering=False, debug=False, num_devices=NCORES)

    P = {}
    def par(name, shape, dt):
        P[name] = nc.declare_dram_parameter(name, list(shape), dt, isOutput=False)
        return P[name]

    xT = par("xT", (D, N), BF)
    xres = par("xres", (ROWS, D), F32)
    wqkvg = par("wqkvg", (D, 512), BF)
    w_out = par("w_out", (D, D), BF)
    wgu = par("wgu", (D, 2 * FFN), BF)
    wdn = par("wdn", (FFN, D), BF)
    bgate = par("bgate", (128, 128), BF)
    pmT_in = par("pmT", (128, 3, 2, 3, 128), BF)
    pmF_in = par("pmF", (128, NT, 2, NFAR), BF)
    ident_in = par("ident", (128, 128), BF)
    y = nc.declare_dram_parameter("y", [ROWS, D], F32, isOutput=True)

    QC, KC, VC, GC = slice(0, 128), slice(128, 256), slice(256, 384), slice(384, 512)

    with TileContext(nc) as tc:
      with (
        tc.tile_pool(name="const", bufs=1) as cp,
        tc.tile_pool(name="dramp", bufs=1, space="DRAM") as dp,
      ):
        pp = tc.alloc_tile_pool(name="persist", bufs=1)
        ident = cp.tile([128, 128], BF)
        nc.sync.dma_start(ident[:], ident_in.ap())
        identF = cp.tile([128, 128], F32)
        nc.scalar.activation(identF[:], ident[:], AF.Copy)
        bg = cp.tile([128, 128], BF)
        nc.sync.dma_start(bg[:], bgate.ap())
        pmT = cp.tile([128, 3, 2, 3, 128], BF)
        nc.sync.dma_start(pmT[:], pmT_in.ap())
        pmF = cp.tile([128, NT, 2, NFAR], BF)
        nc.sync.dma_start(pmF[:], pmF_in.ap())
        epsb = cp.tile([128, 1], F32)
        nc.gpsimd.memset(epsb[:], 1e-6)
        ones = cp.tile([128, 1], BF)
        nc.gpsimd.memset(ones[:], 1.0)

        # persistent activation buffers.  qkv tile index t+2 <-> row tile t;
        # indices 0,1 are zero tiles (band/AV windows read tiles t-2,t-1).
        qkv = pp.tile([128, NT + 2, 384], BF)
        gateb = pp.tile([128, NT, 128], BF)
        qT2 = pp.tile([128, N], BF)                 # (d2, n) transposed q (pre-scaled 1/8)
        kT2 = pp.tile([128, 256 + N], BF)           # zero prefix of 256 cols
        S_far = pp.tile([128, NT, 2, NFAR], BF)
        A_far = pp.tile([128, NT, 2, NFAR], F32)
        far_sum = pp.tile([128, NT, 2], F32)
        snear_row = pp.tile([1, 32 * 128], F32)     # per-(t,h) near alpha sums
        ssum = pp.tile([128, NT, 2], F32)
        rec = pp.tile([128, NT, 2], F32)
        ss_all = pp.tile([128, NT], F32)
        rrms = pp.tile([128, NT], F32)
        navs = pp.tile([128, NT, 2, 64], F32)
        acc_all = pp.tile([128, NT, 2, 64], F32)

        nc.gpsimd.memset(qkv[:, 0:2, :], 0.0)
        nc.gpsimd.memset(kT2[:, 0:256], 0.0)
        nc.gpsimd.memset(S_far[:], 0.0)

        cc_in = dp.tile([N, 128], BF, tag="cc_in")
        cc_out = dp.tile([N, 128], BF, tag="cc_out")

        # ---------- phase B: fused qkv+gate matmul, rrms on eviction ----------
        with (
            tc.tile_pool(name="qph", bufs=1) as qp,
        ):
            psR = tc.alloc_tile_pool(name="psR", bufs=1, space="PSUM")
            wq = qp.tile([128, KD, 512], BF)
            for k in range(KD):
                nc.sync.dma_start(wq[:, k, :], wqkvg.ap()[k * 128:(k + 1) * 128, :])
            xts = qp.tile([128, KD, N], BF)
            for k in range(KD):
                nc.sync.dma_start(xts[:, k, :], xT.ap()[k * 128:(k + 1) * 128, :])
            # rmsnorm1 stats from xT: sumsq over d via ones-matmul on squared tiles
            pss = [psR.tile([1, 512], F32, tag=f"pss{j}", bufs=1, name=f"pss{j}")
                   for j in range(4)]
            for k in range(KD):
                xsq = qp.tile([128, N], BF, tag="xsq", bufs=2)
                nc.scalar.activation(xsq[:], xts[:, k, :], AF.Square)
                for j in range(4):
                    nc.tensor.matmul(pss[j][:], ones[:], xsq[:, j * 512:(j + 1) * 512],
                                     start=(k == 0), stop=(k == KD - 1))
            ssrow = qp.tile([1, N], F32)
            for j in range(4):
                nc.vector.tensor_copy(ssrow[:, j * 512:(j + 1) * 512], pss[j][:])
            ss_dram = dp.tile([1, N], F32, tag="ss_dram")
            nc.sync.dma_start(ss_dram[:], ssrow[:])
            nc.sync.dma_start(
                ss_all[:],
                bass.AP(tensor=ss_dram.tensor, offset=ss_dram.offset,
                        ap=[[1, 128], [128, NT]]))
            srt = cp.tile([128, NT], F32)
            nc.scalar.activation(srt[:], ss_all[:], AF.Sqrt, scale=1.0 / D, bias=epsb[:])
            nc.vector.reciprocal(rrms[:], srt[:])
            psR.release()
            psA = tc.alloc_tile_pool(name="psA", bufs=4, space="PSUM")
            psT = tc.alloc_tile_pool(name="psT", bufs=2, space="PSUM")
            for t in range(NT):
                ps = psA.tile([128, 512], F32, tag="qkvg_ps")
                for k in range(KD):
                    nc.tensor.matmul(ps[:], xts[:, k, t * 128:(t + 1) * 128],
                                     wq[:, k, :], start=(k == 0), stop=(k == KD - 1))
                rr = rrms[:, t:t + 1]
                # q gets the extra 1/sqrt(HD) score scale
                nc.vector.tensor_scalar(qkv[:, t + 2, QC], ps[:, QC], rr,
                                        float(HD) ** -0.5, OP.mult, OP.mult)
                nc.vector.tensor_scalar(qkv[:, t + 2, 128:384], ps[:, 128:384],
                                        rr, None, OP.mult)
                nc.vector.tensor_scalar(gateb[:, t, :], ps[:, GC], rr, None, OP.mult)
                # transposes of q and k for the near-band matmuls
                pq = psT.tile([128, 128], BF, tag="tq")
                nc.tensor.transpose(pq[:], qkv[:, t + 2, QC], ident[:])
                nc.scalar.activation(qT2[:, t * 128:(t + 1) * 128], pq[:], AF.Copy)
                pk = psT.tile([128, 128], BF, tag="tk")
                nc.tensor.transpose(pk[:], qkv[:, t + 2, KC], ident[:])
                nc.scalar.activation(kT2[:, 256 + t * 128:256 + (t + 1) * 128], pk[:], AF.Copy)
            psT.release()
            psA.release()

        # ---------- phase F: far scores + far AV (gpsimd; all tile-aligned) ----
        with tc.tile_pool(name="farp", bufs=2) as fp_:
            for oi, o in enumerate(FAR):
                s = o // 128
                ntl = NT - s
                tmp = fp_.tile([128, NT, 128], BF, tag="ftmp")
                nc.gpsimd.tensor_mul(tmp[:, 0:ntl, :],
                                     qkv[:, s + 2:NT + 2, QC],
                                     qkv[:, 2:NT + 2 - s, KC])
                red_in = tmp[:, 0:ntl, :].rearrange("p t (h d) -> p t h d", h=2)
                with nc.allow_low_precision(reason="scores tolerate bf16"):
                    nc.vector.tensor_reduce(S_far[:, s:NT, :, oi],
                                            red_in, AX.X, OP.add)
            sfb = fp_.tile([128, NT, 2, NFAR], BF)
            nc.gpsimd.tensor_add(sfb[:], S_far[:], pmF[:])
            nc.scalar.activation(A_far[:], sfb[:], AF.Exp)
            nc.vector.tensor_reduce(far_sum[:], A_far[:], AX.X, OP.add)
            nc.gpsimd.memset(acc_all[:], 0.0)

        # ---------- phase E: near band, transposed (scores+softmax+AV on-chip) --
        with (
            tc.tile_pool(name="nearps", bufs=3, space="PSUM") as psS,
            tc.tile_pool(name="nearat", bufs=3) as atp,
            tc.tile_pool(name="nearav", bufs=3) as avp,
            tc.tile_pool(name="psnav", bufs=1, space="PSUM") as psAV,
        ):
            pairs = [(t, h) for t in range(NT) for h in range(2)]
            LAG = 2
            sd_tiles = {}
            for idx in range(len(pairs) + LAG):
                if idx < len(pairs):
                    t, h = pairs[idx]
                    # sdT[j, ck, i] = k[(t+ck-2)*128+j] . q[t*128+i] (zero-padded)
                    sdT = psS.tile([128, 3, 128], F32, tag="sdT")
                    for ck in range(3):
                        nc.tensor.matmul(
                            sdT[:, ck, :],
                            kT2[64 * h:64 * h + 64, (t + ck) * 128:(t + ck + 1) * 128],
                            qT2[64 * h:64 * h + 64, t * 128:(t + 1) * 128],
                            start=True, stop=True)
                    sd_tiles[idx] = sdT
                j = idx - LAG
                if j < 0:
                    continue
                t, h = pairs[j]
                tv = min(t, 2)
                sdT = sd_tiles.pop(j)
                at = atp.tile([128, 3, 128], BF, tag="at")
                nc.vector.tensor_add(at[:], sdT[:], pmT[:, tv, h, :, :])
                ae = atp.tile([128, 3, 128], BF, tag="ae")
                nc.scalar.activation(ae[:], at[:], AF.Exp)
                # softmax denominator: partition-reduce via ones-matmul
                pssum = psAV.tile([1, 128], F32, tag="pssum", bufs=1)
                for ck in range(3):
                    nc.tensor.matmul(pssum[:], ones[:], ae[:, ck, :],
                                     start=(ck == 0), stop=(ck == 2))
                th = 2 * t + h
                nc.scalar.activation(snear_row[:, th * 128:(th + 1) * 128], pssum[:],
                                     AF.Copy)
                # AV numerator, (c, i) orientation; then transpose back
                vc = slice(256 + 64 * h, 256 + 64 * h + 64)
                pav = psAV.tile([64, 128], F32, tag="pav", bufs=2)
                for ck in range(3):
                    nc.tensor.matmul(pav[:], qkv[:, t + ck, vc], ae[:, ck, :],
                                     start=(ck == 0), stop=(ck == 2))
                nav_sb = avp.tile([64, 128], F32, tag="nav_sb")
                nc.scalar.activation(nav_sb[:], pav[:], AF.Copy)
                pnt = psAV.tile([128, 64], F32, tag="pnt", bufs=1)
                nc.tensor.transpose(pnt[:], nav_sb[:], identF[0:64, 0:64])
                nc.scalar.activation(navs[:, t, h, :], pnt[:], AF.Copy)
                # far AV for this (t,h): chained STT, SBUF tile reindex
                for oi, o in enumerate(FAR):
                    s = o // 128
                    if t >= s:
                        nc.vector.scalar_tensor_tensor(
                            acc_all[:, t, h, :],
                            qkv[:, t + 2 - s, vc],
                            A_far[:, t, h, oi:oi + 1],
                            acc_all[:, t, h, :], OP.mult, OP.add)

            # ---------- softmax denominators + og ----------
            sn_dram = dp.tile([1, 32 * 128], F32, tag="sn_dram")
            nc.sync.dma_start(sn_dram[:], snear_row[:])
            nsumT = avp.tile([128, 32], F32, tag="nsumT", bufs=1)
            nc.sync.dma_start(
                nsumT[:],
                bass.AP(tensor=sn_dram.tensor, offset=sn_dram.offset,
                        ap=[[1, 128], [128, 32]]))
            nc.vector.tensor_add(ssum[:], nsumT[:].rearrange("p (t h) -> p t h", h=2),
                                 far_sum[:])
            nc.vector.reciprocal(rec[:], ssum[:])

            with tc.tile_pool(name="ogp", bufs=4) as ogp:
                for t in range(NT):
                    gt = ogp.tile([128, 128], F32, tag="gate")
                    gtr = ogp.tile([128, 128], BF, tag="gtr")
                    nc.vector.tensor_add(gtr[:], gateb[:, t, :], bg[:])
                    nc.scalar.activation(gt[:], gtr[:], AF.Sigmoid)
                    og = ogp.tile([128, 128], BF, tag="og")
                    for h in range(2):
                        comb = ogp.tile([128, 64], F32, tag="comb")
                        nc.gpsimd.tensor_add(comb[:], navs[:, t, h, :], acc_all[:, t, h, :])
                        nc.vector.scalar_tensor_tensor(
                            og[:, 64 * h:64 * h + 64], comb[:],
                            rec[:, t, h:h + 1], gt[:, 64 * h:64 * h + 64],
                            OP.mult, OP.mult)
                    nc.sync.dma_start(cc_in[t * 128:(t + 1) * 128, :], og[:])

        # ---------- phase I: AllToAll + assemble own 256 rows ----------
        pp.release()
        nc.gpsimd.collective_compute(
            "AllToAll", mybir.AluOpType.bypass,
            replica_groups=[list(range(NCORES))],
            ins=[cc_in.opt()], outs=[cc_out.opt()],
        )

        with (
            tc.tile_pool(name="oproj", bufs=1) as op_,
        ):
            psO = tc.alloc_tile_pool(name="psO", bufs=2, space="PSUM")
            psT2 = tc.alloc_tile_pool(name="psT2", bufs=1, space="PSUM")
            ogf = op_.tile([128, 2, D], BF)      # (n-part, nb, d2)
            for r in range(NCORES):
                for b in range(2):
                    nc.sync.dma_start(ogf[:, b, r * 128:(r + 1) * 128],
                                      cc_out[r * ROWS + b * 128:r * ROWS + (b + 1) * 128, :])
            ogfT = op_.tile([128, KD, ROWS], BF)  # (d2-part, k, n)
            for b in range(2):
                for k in range(KD):
                    pt = psT2.tile([128, 128], BF, tag="ot")
                    nc.tensor.transpose(pt[:], ogf[:, b, k * 128:(k + 1) * 128], ident[:])
                    nc.scalar.activation(ogfT[:, k, b * 128:(b + 1) * 128], pt[:], AF.Copy)

            wo = op_.tile([128, KD, D], BF)
            for k in range(KD):
                nc.sync.dma_start(wo[:, k, :], w_out.ap()[k * 128:(k + 1) * 128, :])
            x2 = op_.tile([128, 2, D], F32)
            xr = op_.tile([128, 2, D], F32)
            nc.sync.dma_start(xr[:], xres.ap().rearrange("(b p) c -> p b c", p=128))
            for b in range(2):
                for half in range(2):
                    ps = psO.tile([128, 512], F32, tag="ops")
                    cs = slice(half * 512, (half + 1) * 512)
                    for k in range(KD):
                        nc.tensor.matmul(ps[:], ogfT[:, k, b * 128:(b + 1) * 128],
                                         wo[:, k, cs], start=(k == 0), stop=(k == KD - 1))
                    nc.vector.tensor_add(x2[:, b, cs], ps[:], xr[:, b, cs])

            # ---------- norm2 + transpose ----------
            ss2 = op_.tile([128, 2], F32)
            for b in range(2):
                sq2 = op_.tile([128, D], F32, tag="sq2", bufs=2)
                nc.scalar.activation(sq2[:], x2[:, b, :], AF.Square,
                                     accum_out=ss2[:, b:b + 1])
            srt2 = op_.tile([128, 2], F32)
            nc.scalar.activation(srt2[:], ss2[:], AF.Sqrt, scale=1.0 / D, bias=epsb[:])
            rr2 = op_.tile([128, 2], F32)
            nc.vector.reciprocal(rr2[:], srt2[:])
            xn2 = op_.tile([128, 2, D], BF)
            for b in range(2):
                nc.vector.tensor_scalar(xn2[:, b, :], x2[:, b, :], rr2[:, b:b + 1],
                                        None, OP.mult)
            xn2T = op_.tile([128, KD, ROWS], BF)
            for b in range(2):
                for k in range(KD):
                    pt = psT2.tile([128, 128], BF, tag="xt2")
                    nc.tensor.transpose(pt[:], xn2[:, b, k * 128:(k + 1) * 128], ident[:])
                    nc.scalar.activation(xn2T[:, k, b * 128:(b + 1) * 128], pt[:], AF.Copy)

            # ---------- FFN ----------
            psT2.release()
            psO.release()
            FT = FFN // 128  # 22
            with (
                tc.tile_pool(name="ffnw", bufs=3) as fw,
                tc.tile_pool(name="ffnh", bufs=1) as fh,
                tc.tile_pool(name="psF", bufs=1, space="PSUM") as psF,
            ):
                hT = fh.tile([128, FT, ROWS], BF)
                for m in range(FT):
                    wg_m = fw.tile([128, KD, 128], BF, tag="wg")
                    nc.sync.dma_start(
                        wg_m[:], wgu.ap()[:, m * 128:(m + 1) * 128]
                        .rearrange("(k p) c -> p k c", p=128))
                    wu_m = fw.tile([128, KD, 128], BF, tag="wu")
                    nc.sync.dma_start(
                        wu_m[:], wgu.ap()[:, FFN + m * 128:FFN + (m + 1) * 128]
                        .rearrange("(k p) c -> p k c", p=128))
                    pg = psF.tile([128, ROWS], F32, tag="pg", bufs=2)
                    pu = psF.tile([128, ROWS], F32, tag="pu", bufs=2)
                    for k in range(KD):
                        nc.tensor.matmul(pg[:], wg_m[:, k, :], xn2T[:, k, :],
                                         start=(k == 0), stop=(k == KD - 1))
                    for k in range(KD):
                        nc.tensor.matmul(pu[:], wu_m[:, k, :], xn2T[:, k, :],
                                         start=(k == 0), stop=(k == KD - 1))
                    sg = fw.tile([128, ROWS], F32, tag="sg", bufs=2)
                    nc.scalar.activation(sg[:], pg[:], AF.Silu)
                    nc.vector.tensor_mul(hT[:, m, :], sg[:], pu[:])

                out_sb = fh.tile([128, 2, D], F32)
                pds = [psF.tile([128, 512], F32, tag=f"pd{j}", bufs=1, name=f"pd{j}")
                       for j in range(4)]
                for k2 in range(FT):
                    wd_k = fw.tile([128, D], BF, tag="wdk")
                    nc.sync.dma_start(wd_k[:], wdn.ap()[k2 * 128:(k2 + 1) * 128, :])
                    for b in range(2):
                        for half in range(2):
                            nc.tensor.matmul(
                                pds[b * 2 + half][:],
                                hT[:, k2, b * 128:(b + 1) * 128],
                                wd_k[:, half * 512:(half + 1) * 512],
                                start=(k2 == 0), stop=(k2 == FT - 1))
                for b in range(2):
                    for half in range(2):
                        cs = slice(half * 512, (half + 1) * 512)
                        nc.vector.tensor_add(out_sb[:, b, cs], pds[b * 2 + half][:],
                                             x2[:, b, cs])
                for b in range(2):
                    nc.sync.dma_start(y.ap()[b * 128:(b + 1) * 128, :], out_sb[:, b, :])

    nc.finalize()
    return nc


def _host_prep(inputs):
    x = np.asarray(inputs["x"], np.float32)
    n1 = np.asarray(inputs["norm1_scale"], np.float32)
    n2 = np.asarray(inputs["norm2_scale"], np.float32)
    w_qkv = np.asarray(inputs["w_qkv"], np.float32)
    w_out = np.asarray(inputs["w_out"], np.float32)
    w_gate = np.asarray(inputs["w_gate"], np.float32)
    b_gate = np.asarray(inputs["b_gate"], np.float32)
    pos_bias = np.asarray(inputs["pos_bias"], np.float32)
    w_fg = np.asarray(inputs["w_ffn_gate"], np.float32)
    w_fu = np.asarray(inputs["w_ffn_up"], np.float32)
    w_fd = np.asarray(inputs["w_ffn_down"], np.float32)
    offs = np.asarray(inputs["offsets"], np.int64)
    assert list(offs) == OFFS, "offset set changed; kernel segmentation is stale"

    x2d = np.ascontiguousarray(x.reshape(N, D))
    xT = np.ascontiguousarray(x2d.T.astype(BF16NP))
    wgu = np.ascontiguousarray((np.concatenate([w_fg, w_fu], axis=1)
                                * n2[:, None]).astype(BF16NP))
    wdn_b = np.ascontiguousarray(w_fd.astype(BF16NP))
    w_out_b = np.ascontiguousarray(w_out.astype(BF16NP))
    ident = np.eye(128, dtype=BF16NP)
    wq_s = w_qkv * n1[:, None]
    wg_s = w_gate * n1[:, None]

    tvec = np.arange(N).reshape(NT, 128)
    jj = np.arange(128)
    ii = np.arange(128)
    # o for (j' = ck*128+j, i): o = i + 256 - j'
    o_grid = [ii[None, :] + 256 - (ck * 128 + jj)[:, None] for ck in range(3)]

    in_maps = []
    for c in range(NCORES):
        h0, h1 = 2 * c, 2 * c + 1
        cols = []
        for sec in range(3):  # q, k, v
            for h in (h0, h1):
                cols.append(wq_s[:, sec * D + h * HD: sec * D + (h + 1) * HD])
        cols.append(wg_s[:, c * 128:(c + 1) * 128])
        wqkvg = np.ascontiguousarray(np.concatenate(cols, axis=1).astype(BF16NP))

        # transposed band bias+mask: pmT[j, tv, hh, ck, i]
        pmTc = np.full((128, 3, 2, 3, 128), NEG, np.float32)
        for hh, h in enumerate((h0, h1)):
            for tv in range(3):
                for ck in range(3):
                    o = o_grid[ck]  # (128 j, 128 i)
                    val = np.full((128, 128), NEG, np.float32)
                    for ob in BANDSET:
                        sel = (o == ob) & ((tv * 128 + ii[None, :]) >= ob)
                        if sel.any():
                            val = np.where(sel, pos_bias[OFFS.index(ob), h], val)
                    pmTc[:, tv, hh, ck, :] = val
        # far bias+mask: pmF[i, t, hh, oi]
        pmFc = np.full((128, NT, 2, NFAR), NEG, np.float32)
        for hh, h in enumerate((h0, h1)):
            for oi, o in enumerate(FAR):
                valid = (tvec >= o)  # (NT, 128)
                pmFc[:, :, hh, oi] = np.where(valid.T, pos_bias[OFFS.index(o), h], NEG)
        bgate_b = np.broadcast_to(b_gate[c * 128:(c + 1) * 128], (128, 128))

        in_maps.append({
            "xT": xT,
            "xres": np.ascontiguousarray(x2d[c * ROWS:(c + 1) * ROWS]),
            "wqkvg": wqkvg,
            "w_out": w_out_b,
            "wgu": wgu,
            "wdn": wdn_b,
            "bgate": np.ascontiguousarray(bgate_b.astype(BF16NP)),
            "pmT": np.ascontiguousarray(pmTc.astype(BF16NP)),
            "pmF": np.ascontiguousarray(pmFc.astype(BF16NP)),
            "ident": ident,
        })
    return in_maps


def _get_nc():
    if "nc" not in _CACHE:
        _CACHE["nc"] = _build()
    return _CACHE["nc"]


def kernel(**inputs) -> np.ndarray:
    from concourse import bass_utils
    nc = _get_nc()
    in_maps = _host_prep(inputs)
    res = bass_utils.run_bass_kernel_spmd(
        nc, in_maps, core_ids=list(range(NCORES)), trace=False)
    y = np.concatenate([res.results[c]["y"] for c in range(NCORES)], axis=0)
    return y.reshape(B, N, D).astype(np.float32)


# keep a handle for test.py to run with tracing
def run_traced(inputs, tmpdir=None):
    from concourse import bass_utils
    nc = _get_nc()
    in_maps = _host_prep(inputs)
    res = bass_utils.run_bass_kernel_spmd(
        nc, in_maps, core_ids=list(range(NCORES)), trace=True, tmpdir=tmpdir)
    y = np.concatenate([res.results[c]["y"] for c in range(NCORES)], axis=0)
    return y.reshape(B, N, D).astype(np.float32), res
